# revision 10
# baseline (speedup 1.0000x reference)
"""Trainium2 Bass kernel for nn_MultiHeadAttention_4810363372776 (linear attention).

Sharding: data-parallel over batch (4) x tensor-parallel over head groups (2).
Core i handles batch i//2, heads [8*(i%2), 8*(i%2)+8). Each core computes its
partial output projection; the host sums the two head-group partials per batch.
"""

import functools
import numpy as np

B, S, D, H = 4, 4096, 1024, 16
DK = D // H          # 64
OG = D // 2          # 512 per-core head-group width (8 heads)
NCORES = 8
SCALE = 1.0 / 8.0    # 1/sqrt(DK)
NT = S // 128        # 32 s-tiles


@functools.lru_cache(maxsize=2)
def _build(kv_bias=False):
    import concourse.bass as bass  # noqa: F401
    from concourse import bacc
    import concourse.mybir as mybir
    import concourse.tile as tile
    from concourse.masks import make_identity
    from contextlib import ExitStack

    f32 = mybir.dt.float32
    bf16 = mybir.dt.bfloat16
    fp8 = mybir.dt.float8e4
    DR = mybir.MatmulPerfMode.DoubleRow
    EXP = mybir.ActivationFunctionType.Exp
    COPY = mybir.ActivationFunctionType.Copy
    AXX = mybir.AxisListType.X
    ADD = mybir.AluOpType.add

    nc = bacc.Bacc()

    xq = nc.declare_dram_parameter("xq", [S, D], f32, isOutput=False)
    xk = nc.declare_dram_parameter("xk", [S, D], f32, isOutput=False)
    xv = nc.declare_dram_parameter("xv", [S, D], f32, isOutput=False)
    wqt = nc.declare_dram_parameter("wqt", [D, OG], fp8, isOutput=False)
    wkt = nc.declare_dram_parameter("wkt", [D, OG], fp8, isOutput=False)
    wvt = nc.declare_dram_parameter("wvt", [D, OG], bf16, isOutput=False)
    wot = nc.declare_dram_parameter("wot", [OG, D], bf16, isOutput=False)
    bqsp = nc.declare_dram_parameter("bqs", [128, 4], f32, isOutput=False)
    bkp = nc.declare_dram_parameter("bk", [1, OG], f32, isOutput=False)
    bvp = nc.declare_dram_parameter("bv", [1, OG], f32, isOutput=False)
    bop = nc.declare_dram_parameter("bo", [1, D], f32, isOutput=False)
    maskp = nc.declare_dram_parameter("maskf", [128, NT], f32, isOutput=False)
    out = nc.declare_dram_parameter("out", [S, D], f32, isOutput=True)

    with tile.TileContext(nc) as tc:
        with ExitStack() as ctx:
            singles = ctx.enter_context(tc.tile_pool(name="singles", bufs=1))

            ident = singles.tile([128, 128], bf16)
            nc.vector.memset(ident, 0.0)
            make_identity(nc, ident, nomemset=True)

            wq_sb = singles.tile([128, 8, OG], fp8, tag="wq")
            nc.sync.dma_start(out=wq_sb, in_=wqt[:, :].rearrange("(t p) o -> p t o", p=128))
            wk_sb = singles.tile([128, 8, OG], fp8, tag="wk")
            nc.sync.dma_start(out=wk_sb, in_=wkt[:, :].rearrange("(t p) o -> p t o", p=128))
            wv_sb = singles.tile([128, 8, OG], bf16, tag="wv")
            nc.sync.dma_start(out=wv_sb, in_=wvt[:, :].rearrange("(t p) o -> p t o", p=128))
            wo_sb = singles.tile([128, 4, D], bf16, tag="wo")
            nc.sync.dma_start(out=wo_sb, in_=wot[:, :].rearrange("(t p) o -> p t o", p=128))

            bqs_sb = singles.tile([128, 4], f32, tag="bqs")
            nc.sync.dma_start(out=bqs_sb, in_=bqsp[:, :])
            bo_bc = singles.tile([128, D], f32, tag="bo_bc")
            if kv_bias:
                bk_bc = singles.tile([128, OG], f32, tag="bk_bc")
                bv_bc = singles.tile([128, OG], f32, tag="bv_bc")
            mask_sb = singles.tile([128, NT], f32, tag="mask")
            nc.sync.dma_start(out=mask_sb, in_=maskp[:, :])

            # exp(q_hat * scale), stored [o (4 blocks of 128 = head pairs), s]
            ET = singles.tile([128, 4, S], bf16, tag="ET")
            # block-diag [kv | ksum] per head pair
            kvbd = [singles.tile([128, 130], bf16, tag=f"kvbd{p}", name=f"kvbd{p}") for p in range(4)]

            # ---------------- phase 1 ----------------
            SM = 512
            NU = SM // 128
            with ExitStack() as p1:
                pacc_pool = p1.enter_context(tc.tile_pool(name="pacc", bufs=1, space="PSUM"))
                # two chains per bank; bank-wide has_written clear happens once (st==0, even pair)
                kvps = [pacc_pool.tile([128, 2, 129], f32, tag=f"kvacc{i}", name=f"kvacc{i}") for i in range(2)]
                xin_pool = p1.enter_context(tc.tile_pool(name="xin", bufs=2))
                xt_pool = p1.enter_context(tc.tile_pool(name="xt", bufs=2))
                kvf_pool = p1.enter_context(tc.tile_pool(name="kvf", bufs=4))
                ptr_pool = p1.enter_context(tc.tile_pool(name="ptr", bufs=3, space="PSUM"))
                pkv_pool = p1.enter_context(tc.tile_pool(name="pkv", bufs=3, space="PSUM"))

                pending = []  # (kf, vf, st) deferred kv accumulations

                def flush_kv(pending):
                    kf, vf, pst = pending
                    for p in range(4):
                        nc.tensor.matmul(
                            kvps[p // 2][:, p % 2, 0:129],
                            kf[:, 2 * p:2 * p + 2, :],
                            vf[:, p, 0:129],
                            start=(pst == 0 and p % 2 == 0),
                            stop=(pst == NT - 1),
                            skip_group_check=True,
                        )

                for a in range(S // SM):
                    xq_sb = xin_pool.tile([128, NU, D], bf16, tag="xq")
                    xk_sb = xin_pool.tile([128, NU, D], bf16, tag="xk")
                    xv_sb = xin_pool.tile([128, NU, D], bf16, tag="xv")
                    if a == 0:
                        # split the first loads per s-tile so the PE can start sooner
                        for xs, xd in ((xq_sb, xq), (xk_sb, xk), (xv_sb, xv)):
                            for u in range(NU):
                                nc.gpsimd.dma_start(
                                    out=xs[:, u:u + 1, :],
                                    in_=xd[u * 128:(u + 1) * 128, :].rearrange("(w p) d -> p w d", p=128))
                        if kv_bias:
                            nc.gpsimd.dma_start(out=bk_bc, in_=bkp[:, :].partition_broadcast(128))
                            nc.gpsimd.dma_start(out=bv_bc, in_=bvp[:, :].partition_broadcast(128))
                    else:
                        nc.gpsimd.dma_start(out=xq_sb, in_=xq[a * SM:(a + 1) * SM, :].rearrange("(u p) d -> p u d", p=128))
                        nc.gpsimd.dma_start(out=xk_sb, in_=xk[a * SM:(a + 1) * SM, :].rearrange("(u p) d -> p u d", p=128))
                        nc.gpsimd.dma_start(out=xv_sb, in_=xv[a * SM:(a + 1) * SM, :].rearrange("(u p) d -> p u d", p=128))

                    xqT = xt_pool.tile([128, 8, SM], fp8, tag="xqT")
                    xkT = xt_pool.tile([128, 8, SM], fp8, tag="xkT")
                    xvT = xt_pool.tile([128, 8, SM], bf16, tag="xvT")

                    for u in range(NU):
                        st = a * NU + u

                        # transpose x tiles: [s,d] -> [d,s] via PE, evacuate to bf16
                        for x_sb, x_t, eng in (
                            (xq_sb, xqT, nc.vector),
                            (xk_sb, xkT, nc.scalar),
                            (xv_sb, xvT, nc.vector),
                        ):
                            for b2 in range(2):
                                ptr = ptr_pool.tile([128, 512], bf16, tag="tr")
                                for j in range(4):
                                    db = b2 * 4 + j
                                    nc.tensor.transpose(
                                        ptr[:, j * 128:(j + 1) * 128],
                                        x_sb[:, u, db * 128:(db + 1) * 128],
                                        ident,
                                    )
                                dst = x_t[:, b2 * 4:(b2 + 1) * 4, u * 128:(u + 1) * 128]
                                src = ptr.rearrange("p (j s) -> p j s", j=4)
                                if eng is nc.vector:
                                    nc.vector.tensor_copy(dst, src)
                                else:
                                    nc.scalar.copy(out=dst, in_=src)

                        # k projection
                        pk = pkv_pool.tile([128, OG], f32, tag="pkv")
                        for t2 in range(4):
                            nc.tensor.matmul(pk, xkT[:, 2 * t2:2 * t2 + 2, u * 128:(u + 1) * 128],
                                             wk_sb[:, 2 * t2:2 * t2 + 2, :],
                                             start=(t2 == 0), stop=(t2 == 3), perf_mode=DR)
                        if kv_bias:
                            nc.vector.tensor_add(pk, pk, bk_bc)
                        ek = kvf_pool.tile([128, OG], bf16, tag="ek")
                        nc.scalar.activation(ek, pk, EXP, scale=SCALE)
                        rows = kvf_pool.tile([128, 8], f32, tag="rows")
                        nc.vector.tensor_reduce(rows, ek.rearrange("p (h e) -> p h e", h=8), axis=AXX, op=ADD)
                        nc.vector.reciprocal(rows, rows)
                        nc.vector.tensor_scalar_mul(rows, rows, mask_sb[:, st:st + 1])
                        kf = kvf_pool.tile([128, 8, DK], bf16, tag="kf")
                        nc.vector.tensor_mul(
                            kf,
                            ek.rearrange("p (h e) -> p h e", h=8),
                            rows[:, :, None].to_broadcast([128, 8, DK]),
                        )

                        # v projection
                        pv = pkv_pool.tile([128, OG], f32, tag="pkv")
                        for t in range(8):
                            nc.tensor.matmul(pv, xvT[:, t, u * 128:(u + 1) * 128], wv_sb[:, t, :], start=(t == 0), stop=(t == 7))
                        if kv_bias:
                            nc.vector.tensor_add(pv, pv, bv_bc)
                        vf = kvf_pool.tile([128, 4, 130], bf16, tag="vf")
                        nc.scalar.activation(vf[:, :, 0:128], pv.rearrange("p (j s) -> p j s", j=4), COPY, scale=mask_sb[:, st:st + 1])
                        nc.vector.memset(vf[:, :, 128:129], 1.0)

                        # deferred kv accumulation, two s-tiles behind
                        pending.append((kf, vf, st))
                        if len(pending) > 2:
                            flush_kv(pending.pop(0))

                    # q projection for the macro, output transposed [o, s]
                    for ob in range(4):
                        pq = pkv_pool.tile([128, SM], f32, tag="pkv")
                        for t2 in range(4):
                            nc.tensor.matmul(pq, wq_sb[:, 2 * t2:2 * t2 + 2, ob * 128:(ob + 1) * 128],
                                             xqT[:, 2 * t2:2 * t2 + 2, :],
                                             start=(t2 == 0), stop=(t2 == 3), perf_mode=DR)
                        nc.scalar.activation(ET[:, ob, a * SM:(a + 1) * SM], pq, EXP, bias=bqs_sb[:, ob:ob + 1], scale=SCALE)

                for pend in pending:
                    flush_kv(pend)

                # build block-diag [kv | ksum] tiles (bf16)
                for p in range(4):
                    ps = kvps[p // 2][:, p % 2]
                    nc.vector.memset(kvbd[p], 0.0)
                    nc.vector.tensor_copy(kvbd[p][0:64, 0:64], ps[0:64, 0:64])
                    nc.vector.tensor_copy(kvbd[p][0:64, 64:65], ps[0:64, 128:129])
                    nc.vector.tensor_copy(kvbd[p][64:128, 65:129], ps[64:128, 64:128])
                    nc.vector.tensor_copy(kvbd[p][64:128, 129:130], ps[64:128, 128:129])

            nc.gpsimd.dma_start(out=bo_bc, in_=bop[:, :].partition_broadcast(128))

            # ---------------- phase 2 ----------------
            # stages per s-tile: num -> (DVE) ctx -> (PE) ctxT -> (ACT) evac -> (PE) out-proj
            # software-pipelined: ctxT lags one tile, out-proj lags two.
            with ExitStack() as p2s:
                p2 = p2s.enter_context(tc.tile_pool(name="p2", bufs=3))
                pnum_pool = p2s.enter_context(tc.tile_pool(name="pnum", bufs=2, space="PSUM"))
                pct_pool = p2s.enter_context(tc.tile_pool(name="pct", bufs=2, space="PSUM"))
                po_pool = p2s.enter_context(tc.tile_pool(name="po", bufs=2, space="PSUM"))

                ctx_q = {}   # st -> ctx tile
                ctxT_q = {}  # st -> ctxT tile

                def stage_num(st):
                    s0 = st * 128
                    pnums = [pnum_pool.tile([128, 2, 130], f32, tag=f"pnum{i}", name=f"pnum{i}") for i in range(2)]
                    for p in range(4):
                        nc.tensor.matmul(pnums[p // 2][:, p % 2, :], ET[:, p, s0:s0 + 128], kvbd[p], start=True, stop=True)
                    ctxs = p2.tile([128, OG], bf16, tag="ctx", name="ctxs")
                    for i in range(2):
                        pn4 = pnums[i].rearrange("p j (two c) -> p (j two) c", two=2)  # [128, 4, 65]
                        r4 = p2.tile([128, 4, 1], f32, tag="r", name="r4")
                        nc.vector.reciprocal(r4, pn4[:, :, 64:65])
                        ctx4 = ctxs[:, i * 256:(i + 1) * 256].rearrange("p (j c) -> p j c", c=64)
                        nc.vector.tensor_mul(ctx4, pn4[:, :, 0:64], r4.to_broadcast([128, 4, 64]))
                    ctx_q[st] = ctxs

                def stage_ctxT(st):
                    ctxs = ctx_q.pop(st)
                    pct = pct_pool.tile([128, 512], bf16, tag="pct", name="pct")
                    for eb in range(4):
                        nc.tensor.transpose(pct[:, eb * 128:(eb + 1) * 128], ctxs[:, eb * 128:(eb + 1) * 128], ident)
                    ctxT = p2.tile([128, 4, 128], bf16, tag="ctxT", name="ctxT")
                    nc.scalar.copy(out=ctxT, in_=pct.rearrange("p (j s) -> p j s", j=4))
                    ctxT_q[st] = ctxT

                def stage_oproj(st):
                    s0 = st * 128
                    ctxT = ctxT_q.pop(st)
                    outsb = p2.tile([128, D], f32, tag="outsb", name="outsb")
                    for half in range(2):
                        po = po_pool.tile([128, 512], f32, tag="po", name="po")
                        for eb in range(4):
                            nc.tensor.matmul(po, ctxT[:, eb, :], wo_sb[:, eb, half * 512:(half + 1) * 512], start=(eb == 0), stop=(eb == 3))
                        nc.vector.tensor_add(outsb[:, half * 512:(half + 1) * 512], po, bo_bc[:, half * 512:(half + 1) * 512])
                    nc.sync.dma_start(out=out[s0:s0 + 128, :], in_=outsb)

                for st in range(NT):
                    stage_num(st)
                    if st >= 1:
                        stage_ctxT(st - 1)
                    if st >= 2:
                        stage_oproj(st - 2)
                stage_ctxT(NT - 1)
                stage_oproj(NT - 2)
                stage_oproj(NT - 1)

    nc.compile()
    return nc


_LAST_RESULT = None


def kernel(q, k, v, mask, Wq, bq, Wk, bk, Wv, bv, Wo, bo):
    global _LAST_RESULT
    import ml_dtypes
    from concourse.bass_utils import run_bass_kernel_spmd

    q = np.asarray(q, np.float32)
    k = np.asarray(k, np.float32)
    v = np.asarray(v, np.float32)
    mask = np.asarray(mask)
    Wq = np.asarray(Wq, np.float32)
    Wk = np.asarray(Wk, np.float32)
    Wv = np.asarray(Wv, np.float32)
    Wo = np.asarray(Wo, np.float32)
    bq = np.asarray(bq, np.float32)
    bk = np.asarray(bk, np.float32)
    bv = np.asarray(bv, np.float32)
    bo = np.asarray(bo, np.float32)

    nc = _build(bool(np.any(bk) or np.any(bv)))

    bf = ml_dtypes.bfloat16
    f8 = ml_dtypes.float8_e4m3
    in_maps = []
    for core in range(NCORES):
        b, g = core // 2, core % 2
        sl = slice(g * OG, (g + 1) * OG)
        maskf = mask[b, 0, 0, :].astype(np.float32).reshape(NT, 128).T.copy()
        in_maps.append({
            "xq": np.ascontiguousarray(q[b]),
            "xk": np.ascontiguousarray(k[b]),
            "xv": np.ascontiguousarray(v[b]),
            "wqt": np.ascontiguousarray(Wq[sl, :].T).astype(f8),
            "wkt": np.ascontiguousarray(Wk[sl, :].T).astype(f8),
            "wvt": np.ascontiguousarray(Wv[sl, :].T).astype(bf),
            "wot": np.ascontiguousarray(Wo[:, sl].T).astype(bf),
            "bqs": np.ascontiguousarray((bq[sl] * SCALE).reshape(4, 128).T),
            "bk": bk[sl].reshape(1, OG).copy(),
            "bv": bv[sl].reshape(1, OG).copy(),
            "bo": (bo if g == 0 else np.zeros_like(bo)).reshape(1, D).copy(),
            "maskf": maskf,
        })

    res = run_bass_kernel_spmd(nc, in_maps, list(range(NCORES)))
    _LAST_RESULT = res

    outp = np.empty((B, S, D), np.float32)
    for b in range(B):
        outp[b] = res.results[2 * b]["out"] + res.results[2 * b + 1]["out"]
    return outp
